# revision 50
# baseline (speedup 1.0000x reference)
"""Distributed Trainium2 kernel for decode-style multi-head attention.

Shape: B=8, S=16, H=32, D=64, HID=2048, PAST=4096 (T=4112 after concat).
Sharding: tensor-parallel over heads — each of 8 cores owns 4 heads:
  wq/wk/wv row-sharded (output features), wo column-sharded (input features),
  past KV naturally per-head; partial out-proj summed with chunked
  ReduceScatters (each core returns only its 16-row shard; the host
  assembles the full output from all 8 cores).

Per-core dataflow (all matmuls out = lhsT.T @ rhs, contract on partitions;
every matmul operand is kept at base partition 0 — base-64 operands fault
on this runtime; partition moves go through SBUF->SBUF DMA instead):
  - x / weight shards cast f32->bf16 (Pool/DVE), PE-transposed in bf16
    (1 cyc/row) -> xT, wqT/wkT/wvT [hid,256], woT [256,2048].
  - projections: qT/kT [256,128] (head-dim major), v [128,256] (token major).
  - per (b,h) pair: KV DMA'd 2KB-interleaved (partition p <- tokens
    {8p..8p+7} of each 1024-token group; consecutive 64-col slices are
    128-token tiles), K cast f32->bf16 on GPSIMD, K tiles PE-transposed in
    bf16 -> kT [64,128] tiles (base 0), PSUM->SBUF extracts on DVE (2-byte
    2x mode); scores^T = stationary kT x moving qT -> PSUM [128tok,16];
    exp on ACT -> probsT bf16; out2^T accumulated as stationary [v|1] bf16
    x moving probsT -> PSUM [65,16] (row 64 = denom); normalize via DVE
    reciprocal + gpsimd partition_broadcast + DVE multiply into attnS.
  - the emission is software-pipelined three stages deep (pre / score /
    norm) so no engine sequencer holds a cross-engine wait that blocks the
    next pair's early work, and the KV stream never stalls on tile reuse.
  - chunked out-proj (batch chunks 3/2/2/1 so the last chunk is small)
    contracts straight out of attnS against a per-head woT2 layout (no
    rebase DMAs); bf16 partials -> one wide cc_in DMA -> per-chunk
    ReduceScatter overlapped with the stream; the tail is just the last
    (16-row) RS plus the final output copy. Output is bf16 (the host casts
    back to f32 when assembling).
"""

import os

import numpy as np

import concourse.bass as bass
import concourse.mybir as mybir
import concourse.tile as tile
from concourse import bacc
from concourse.masks import make_identity
from concourse.bass_utils import run_bass_kernel_spmd

F32 = mybir.dt.float32
BF16 = mybir.dt.bfloat16

B, S, H, D = 8, 16, 32, 64
HID = H * D            # 2048
PAST = 4096
NCORES = 8
HLOC = H // NCORES     # 4 heads per core
SH = HLOC * D          # 256 local head dims
NTOK = B * S           # 128 query tokens
NT = PAST // 128       # 32 full KV tiles (8-token interleave)
SCALE = 1.0 / float(np.sqrt(D))
EXP = mybir.ActivationFunctionType.Exp

# batch chunks for the out-proj / ReduceScatter pipeline: the last chunk is a
# single batch so the post-stream tail is one small RS.
# chunk: (last_batch, row_start, row_end, out_row_start)
CHUNKS = [
    (2, 0, 48, 0),
    (4, 48, 80, 6),
    (6, 80, 112, 10),
    (7, 112, 128, 14),
]


def build_nc():
    skip_cc = os.environ.get("SKIP_CC", "0") == "1"
    kvb = int(os.environ.get("KVB", "5"))
    sbb = int(os.environ.get("SBB", "3"))
    nc = bacc.Bacc(None, target_bir_lowering=False, debug=False, num_devices=NCORES)

    hid_e = nc.declare_dram_parameter("hidden", [NTOK, HID], F32, isOutput=False)
    wq_e = nc.declare_dram_parameter("wq", [SH, HID], F32, isOutput=False)
    wk_e = nc.declare_dram_parameter("wk", [SH, HID], F32, isOutput=False)
    wv_e = nc.declare_dram_parameter("wv", [SH, HID], F32, isOutput=False)
    wo_e = nc.declare_dram_parameter("wo", [HID, SH], F32, isOutput=False)
    pk_e = nc.declare_dram_parameter("pk", [B, HLOC, PAST, D], F32, isOutput=False)
    pv_e = nc.declare_dram_parameter("pv", [B, HLOC, PAST, D], F32, isOutput=False)
    out_e = nc.declare_dram_parameter("out", [16, HID], BF16, isOutput=True)

    cc_in = nc.dram_tensor("cc_in", [NTOK, HID], BF16)
    cc_out = nc.dram_tensor("cc_out", [16, HID], BF16)

    with tile.TileContext(nc) as tc:
        with (
            tc.tile_pool(name="const", bufs=1) as constp,
            tc.tile_pool(name="pers", bufs=1) as pers,
            tc.tile_pool(name="wload", bufs=2) as wload,
            tc.tile_pool(name="kvload", bufs=kvb) as kvload,
            tc.tile_pool(name="kbfp", bufs=sbb) as kbfp,
            tc.tile_pool(name="ktp", bufs=sbb) as ktp,
            tc.tile_pool(name="vbfp", bufs=sbb) as vbfp,
            tc.tile_pool(name="probsp", bufs=sbb) as probsp,
            tc.tile_pool(name="finp", bufs=sbb) as finp,
            tc.tile_pool(name="ochp", bufs=1) as ochp,
            tc.tile_pool(name="psX", bufs=int(os.environ.get("PSX", "3")), space="PSUM") as psX,
            tc.tile_pool(name="psB", bufs=int(os.environ.get("PSB", "2")), space="PSUM") as psB,
            tc.tile_pool(name="psC", bufs=int(os.environ.get("PSC", "3")), space="PSUM") as psC,
        ):
            ident_bf = constp.tile([128, 128], BF16, tag="idb")
            make_identity(nc, ident_bf[:, :])

            # persistent per-core tensors
            xT = pers.tile([128, 16 * 128], BF16, tag="xT")
            wqT = pers.tile([128, 16 * 256], BF16, tag="wqT")
            wkT = pers.tile([128, 16 * 256], BF16, tag="wkT")
            wvT = pers.tile([128, 16 * 256], BF16, tag="wvT")
            woT2 = pers.tile([64, 4 * 2048], BF16, tag="woT2")
            qstage = pers.tile([128, 256], BF16, tag="qstage")
            kstage = pers.tile([128, 256], BF16, tag="kstage")
            qT2 = pers.tile([64, 4 * 128], BF16, tag="qT2")       # [d, hl*128+(b,s)]
            kTn = pers.tile([64, 4 * 128], BF16, tag="kTn")
            vn2 = pers.tile([16, 32 * 65], BF16, tag="vn2")
            attnS = pers.tile([64, 4 * 128], BF16, tag="attnS")   # normalized out2^T

            def load_kv(b, hl, nspl=2):
                kb = kvload.tile([128, 2048], F32, tag="kbuf")
                vb = kvload.tile([128, 2048], F32, tag="vbuf")
                ctok = PAST // nspl          # tokens per split
                ccol = 2048 // nspl          # sbuf cols per split
                gs = max(ctok // 1024, 1)    # 1024-token groups per split
                # partition p <- tokens {8p..8p+7} of each 1024-token group:
                # 2KB contiguous DMA runs; consecutive 64-col slices are still
                # 128-token tiles (for any nspl), and V uses the identical
                # interleave so probsT/v token slots stay consistent.
                for hv in range(nspl):
                    nc.sync.dma_start(
                        out=kb[:, hv * ccol:(hv + 1) * ccol].rearrange(
                            "p (g eight d) -> p g eight d", g=gs, eight=8, d=64),
                        in_=pk_e[b, hl, hv * ctok:(hv + 1) * ctok, :].rearrange(
                            "(g p eight) d -> p g eight d", g=gs, p=128, eight=8
                        ),
                    )
                    nc.sync.dma_start(
                        out=vb[:, hv * ccol:(hv + 1) * ccol].rearrange(
                            "p (g eight d) -> p g eight d", g=gs, eight=8, d=64),
                        in_=pv_e[b, hl, hv * ctok:(hv + 1) * ctok, :].rearrange(
                            "(g p eight) d -> p g eight d", g=gs, p=128, eight=8
                        ),
                    )
                return kb, vb

            pairs = [(b, hl) for b in range(B) for hl in range(HLOC)]

            # prefetch the first pairs' KV ahead of the weight loads so the
            # DMA queue starts on the big stream immediately
            PREF = int(os.environ.get("PREF", "4"))
            prefetch = {}
            for jp in range(PREF):
                prefetch[jp] = load_kv(*pairs[jp])

            # ---------------- pipeline stages ----------------
            def stage_pre(i):
                b, hl = pairs[i]
                nspl = 4 if b == B - 1 else 2
                if i in prefetch:
                    kb, vb = prefetch.pop(i)
                else:
                    kb, vb = load_kv(b, hl, nspl)
                # K cast f32 -> bf16, halves on Pool and DVE in parallel so
                # the per-pair serial chain stays short; split along the DMA
                # splits so transposes start as soon as the first split lands
                kbf = kbfp.tile([128, 2048], BF16, tag="kbf")
                cw = 2048 // nspl
                for hv in range(nspl):
                    eng = nc.gpsimd if hv % 2 == 0 else nc.vector
                    eng.tensor_copy(
                        kbf[:, hv * cw:(hv + 1) * cw], kb[:, hv * cw:(hv + 1) * cw]
                    )
                # v: cast + restride 64 -> 65 cols (ones in col 64) on ACT
                vbf = vbfp.tile([128, 32 * 65], BF16, tag="vbf")
                vbfv = vbf[:, :].rearrange("p (t c) -> p t c", t=32, c=65)
                vbv = vb[:, :].rearrange("p (t d) -> p t d", t=32, d=64)
                nc.vector.memset(vbfv[:, :, 64:65], 1.0)
                tw = 32 // nspl
                for hv in range(nspl):
                    nc.scalar.copy(
                        vbfv[:, hv * tw:(hv + 1) * tw, 0:64],
                        vbv[:, hv * tw:(hv + 1) * tw, :],
                    )
                # K tiles -> kT [64, 32*128] bf16, all at partitions 0:64
                kt = ktp.tile([64, 32 * 128], BF16, tag="kt")
                for gg in range(8):
                    ps = psX.tile([128, 512], BF16, tag="trp")
                    for j in range(4):
                        t = gg * 4 + j
                        nc.tensor.transpose(
                            ps[0:64, j * 128:(j + 1) * 128],
                            kbf[:, t * 64:(t + 1) * 64],
                            ident_bf[:, :],
                        )
                    nc.vector.tensor_copy(kt[:, gg * 512:(gg + 1) * 512], ps[0:64, :])
                return kt, vbf

            def stage_score(i, kt, vbf):
                b, hl = pairs[i]
                pidx = hl * 8 + b
                pt = probsp.tile([128, 544], BF16, tag="pt")
                qsl = qT2[:, hl * 128 + b * 16: hl * 128 + (b + 1) * 16]
                knsl = kTn[:, hl * 128 + b * 16: hl * 128 + (b + 1) * 16]
                for t16 in range(2):
                    ps_sc = psB.tile([128, 256], F32, tag="sc")
                    for j in range(16):
                        t = t16 * 16 + j
                        nc.tensor.matmul(
                            ps_sc[:, j * 16:(j + 1) * 16],
                            lhsT=kt[:, t * 128:(t + 1) * 128],
                            rhs=qsl,
                            start=True,
                            stop=True,
                        )
                    nc.scalar.activation(
                        pt[:, t16 * 256:(t16 + 1) * 256], ps_sc[:, :], EXP
                    )
                ps_sc = psB.tile([128, 256], F32, tag="sc")
                nc.tensor.matmul(
                    ps_sc[0:16, 0:16], lhsT=knsl, rhs=qsl, start=True, stop=True
                )
                nc.scalar.activation(pt[0:16, 512:528], ps_sc[0:16, 0:16], EXP)
                # out2^T accumulation [65, 16]; row 64 = denom
                po = psC.tile([65, 16], F32, tag="out2")
                for t in range(32):
                    nc.tensor.matmul(
                        po[:, :],
                        lhsT=vbf[:, t * 65:(t + 1) * 65],
                        rhs=pt[:, t * 16:(t + 1) * 16],
                        start=(t == 0),
                        stop=False,
                    )
                nc.tensor.matmul(
                    po[:, :],
                    lhsT=vn2[0:16, pidx * 65:(pidx + 1) * 65],
                    rhs=pt[0:16, 512:528],
                    start=False,
                    stop=True,
                )
                return po

            def stage_norm(i, po):
                b, hl = pairs[i]
                rec1 = finp.tile([1, 16], F32, tag="rec")
                nc.vector.reciprocal(rec1[:, :], po[64:65, :])
                recb = finp.tile([64, 16], F32, tag="recb")
                nc.gpsimd.partition_broadcast(recb[:, :], rec1[:, :])
                nc.vector.tensor_tensor(
                    attnS[:, hl * 128 + b * 16: hl * 128 + (b + 1) * 16],
                    po[0:64, :],
                    recb[:, :],
                    mybir.AluOpType.mult,
                )

            def stage_chunk(chunk):
                bl, c0, c1, o0 = chunk
                nr = (c1 - c0) // 8
                o1 = o0 + nr
                och = ochp.tile([48, 2048], BF16, tag="och")
                for n in range(4):
                    pso = psC.tile([c1 - c0, 512], F32, tag="out2")
                    for hl2 in range(4):
                        nc.tensor.matmul(
                            pso[:, :],
                            lhsT=attnS[:, hl2 * 128 + c0: hl2 * 128 + c1],
                            rhs=woT2[:, hl2 * 2048 + n * 512: hl2 * 2048 + (n + 1) * 512],
                            start=(hl2 == 0),
                            stop=(hl2 == 3),
                        )
                    with nc.allow_low_precision(reason="bf16 partials for RS"):
                        nc.scalar.copy(
                            och[0:c1 - c0, n * 512:(n + 1) * 512], pso[:, :]
                        )
                nc.scalar.dma_start(
                    out=(out_e[o0:o1, :] if skip_cc else cc_in[c0:c1, :]),
                    in_=(och[0:nr, :] if skip_cc else och[0:c1 - c0, :]),
                )
                if not skip_cc:
                    nc.gpsimd.collective_compute(
                        "ReduceScatter",
                        mybir.AluOpType.add,
                        replica_groups=[list(range(NCORES))],
                        ins=[cc_in[c0:c1, :].opt()],
                        outs=[cc_out[o0:o1, :].opt()],
                    )

            wcp = [0]

            def wcopy(dst, srcv):
                wcp[0] += 1
                nc.scalar.copy(dst, srcv)

            # pre-stages of the prefetched pairs come first so their Pool
            # casts and PE transposes interleave with the weight setup
            pre_st = {}
            for i in range(min(PREF, 2)):
                pre_st[i] = stage_pre(i)

            # ---------- x load + cast + transpose (all-bf16 PE path) ----------
            def wcast2(dstt, srct):
                nc.gpsimd.tensor_copy(dstt[:, 0:1024], srct[:, 0:1024])
                nc.vector.tensor_copy(dstt[:, 1024:2048], srct[:, 1024:2048])

            xsb = wload.tile([128, 2048], F32, tag="wnat")
            nc.scalar.dma_start(out=xsb[:, :], in_=hid_e[:, :])
            xsbb = kbfp.tile([128, 2048], BF16, tag="kbf")
            wcast2(xsbb, xsb)
            for r4 in range(4):
                ps = psX.tile([128, 512], BF16, tag="trp")
                for j in range(4):
                    r = r4 * 4 + j
                    nc.tensor.transpose(
                        ps[:, j * 128:(j + 1) * 128],
                        xsbb[:, r * 128:(r + 1) * 128],
                        ident_bf[:, :],
                    )
                wcopy(xT[:, r4 * 512:(r4 + 1) * 512], ps[:, :])

            # ---------- wq/wk/wv transposes ----------
            for w_e, dst in ((wq_e, wqT), (wk_e, wkT), (wv_e, wvT)):
                for p in range(2):
                    wn = wload.tile([128, 2048], F32, tag="wnat")
                    nc.scalar.dma_start(out=wn[:, :], in_=w_e[p * 128:(p + 1) * 128, :])
                    wnb = kbfp.tile([128, 2048], BF16, tag="kbf")
                    wcast2(wnb, wn)
                    for r4 in range(4):
                        ps = psX.tile([128, 512], BF16, tag="trp")
                        for j in range(4):
                            r = r4 * 4 + j
                            nc.tensor.transpose(
                                ps[:, j * 128:(j + 1) * 128],
                                wnb[:, r * 128:(r + 1) * 128],
                                ident_bf[:, :],
                            )
                        dview = dst[:, :].rearrange("q (r c) -> q r c", r=16, c=256)
                        wcopy(
                            dview[:, r4 * 4:(r4 + 1) * 4, p * 128:(p + 1) * 128],
                            ps[:, :].rearrange("q (j c) -> q j c", j=4, c=128),
                        )

            # ---------- wo transposes: woT2[d, hl*2048 + n] = wo[n, hl*64+d]
            for hhalf in range(2):
                wn = wload.tile([128, 2048], F32, tag="wnat")
                nc.scalar.dma_start(
                    out=wn[:, :],
                    in_=wo_e[:, :].rearrange("(rr p) c -> p rr c", p=128)[
                        :, hhalf * 8:(hhalf + 1) * 8, :
                    ],
                )
                wnb = kbfp.tile([128, 2048], BF16, tag="kbf")
                wcast2(wnb, wn)
                for hl in range(4):
                    for rr4 in range(2):
                        ps = psX.tile([128, 512], BF16, tag="trp")
                        for j in range(4):
                            rr_rel = rr4 * 4 + j
                            nc.tensor.transpose(
                                ps[0:64, j * 128:(j + 1) * 128],
                                wnb[:, rr_rel * 256 + hl * 64: rr_rel * 256 + (hl + 1) * 64],
                                ident_bf[:, :],
                            )
                        base = hl * 2048 + (hhalf * 8 + rr4 * 4) * 128
                        if wcp[0] % 2 == 0:
                            nc.vector.tensor_copy(woT2[:, base: base + 512], ps[0:64, :])
                        else:
                            nc.scalar.copy(woT2[:, base: base + 512], ps[0:64, :])
                        wcp[0] += 1

            # ---------- projections ----------
            for wTsrc, stg, scl in ((wqT, qstage, SCALE), (wkT, kstage, 1.0)):
                for p in range(2):
                    ps = psB.tile([128, 256], F32, tag="sc")
                    for r in range(16):
                        nc.tensor.matmul(
                            ps[:, 0:128],
                            lhsT=wTsrc[:, r * 256 + p * 128: r * 256 + (p + 1) * 128],
                            rhs=xT[:, r * 128:(r + 1) * 128],
                            start=(r == 0),
                            stop=(r == 15),
                        )
                    if scl != 1.0:
                        nc.scalar.mul(stg[:, p * 128:(p + 1) * 128], ps[:, 0:128], scl)
                    else:
                        nc.scalar.copy(stg[:, p * 128:(p + 1) * 128], ps[:, 0:128])
            # re-base to [64, hl*128 + (b,s)] layout
            for hl in range(4):
                p, hf = hl // 2, hl % 2
                nc.gpsimd.dma_start(
                    out=qT2[:, hl * 128:(hl + 1) * 128],
                    in_=qstage[hf * 64:(hf + 1) * 64, p * 128:(p + 1) * 128],
                )
                nc.gpsimd.dma_start(
                    out=kTn[:, hl * 128:(hl + 1) * 128],
                    in_=kstage[hf * 64:(hf + 1) * 64, p * 128:(p + 1) * 128],
                )

            # v projection [128 tok, 256]
            psv = psB.tile([128, 256], F32, tag="sc")
            for r in range(16):
                nc.tensor.matmul(
                    psv[:, :],
                    lhsT=xT[:, r * 128:(r + 1) * 128],
                    rhs=wvT[:, r * 256:(r + 1) * 256],
                    start=(r == 0),
                    stop=(r == 15),
                )
            vn_sb = finp.tile([128, 256], BF16, tag="vnsb")
            nc.scalar.copy(vn_sb[:, :], psv[:, :])
            vn2v = vn2[:, :].rearrange("s (pr c) -> s pr c", pr=32, c=65)
            nc.vector.memset(vn2v[:, :, 64:65], 1.0)
            for b2 in range(8):
                nc.gpsimd.dma_start(
                    out=vn2[:, :].rearrange("s (hl b c) -> s hl b c",
                                            hl=4, b=8, c=65)[:, :, b2, 0:64],
                    in_=vn_sb[b2 * 16:(b2 + 1) * 16, :].rearrange(
                        "s (hl d) -> s hl d", hl=4
                    ),
                )

            # ------- main attention loop (3-stage software pipeline) -------
            chunk_by_batch = {bl: (bl, c0, c1, o0) for (bl, c0, c1, o0) in CHUNKS}
            NP = len(pairs)
            po_st = {}

            def norm_and_chunk(j):
                stage_norm(j, po_st[j])
                bj, hlj = pairs[j]
                if hlj == HLOC - 1 and bj in chunk_by_batch:
                    stage_chunk(chunk_by_batch[bj])

            # norms run 2 pairs behind the front; the chunk-gating norm of
            # each (b, HLOC-1) pair runs only 1 behind so its chunk's
            # ReduceScatter fires as early as possible (the collective device
            # serializes RS's, so a late RS delays the final one).
            done_norm = set()

            def norm_upto(j):
                for k in range(j + 1):
                    if k not in done_norm and k in po_st:
                        done_norm.add(k)
                        norm_and_chunk(k)

            po_st[0] = stage_score(0, *pre_st.pop(0))
            for i in range(1, NP):
                if i not in pre_st:
                    pre_st[i] = stage_pre(i)
                po_st[i] = stage_score(i, *pre_st.pop(i))
                if pairs[i - 1][1] == HLOC - 1 and pairs[i - 1][0] >= 5:
                    norm_upto(i - 1)
                elif i >= 2:
                    norm_upto(i - 2)
                if i == NP - 1:
                    norm_upto(NP - 1)

            if not skip_cc:
                nc.sync.dma_start(out=out_e[0:14, :], in_=cc_out[0:14, :])
                nc.sync.dma_start(out=out_e[14:16, :], in_=cc_out[14:16, :])

    nc.compile()
    return nc


_CACHE = {}


def _get_nc():
    if "nc" not in _CACHE:
        _CACHE["nc"] = build_nc()
    return _CACHE["nc"]


def make_in_maps(hidden_states, past_k, past_v, wq, wk, wv, wo):
    x = np.ascontiguousarray(np.asarray(hidden_states, np.float32).reshape(NTOK, HID))
    wq = np.asarray(wq, np.float32)
    wk = np.asarray(wk, np.float32)
    wv = np.asarray(wv, np.float32)
    wo = np.asarray(wo, np.float32)
    past_k = np.asarray(past_k, np.float32)
    past_v = np.asarray(past_v, np.float32)
    in_maps = []
    for c in range(NCORES):
        rs = slice(c * SH, (c + 1) * SH)
        in_maps.append({
            "hidden": x,
            "wq": np.ascontiguousarray(wq[rs, :]),
            "wk": np.ascontiguousarray(wk[rs, :]),
            "wv": np.ascontiguousarray(wv[rs, :]),
            "wo": np.ascontiguousarray(wo[:, rs]),
            "pk": np.ascontiguousarray(past_k[:, c * HLOC:(c + 1) * HLOC]),
            "pv": np.ascontiguousarray(past_v[:, c * HLOC:(c + 1) * HLOC]),
        })
    return in_maps


def assemble_out(results):
    # each core's "out" rows are its ReduceScatter shards: for each chunk of
    # rows [c0:c1), core c holds the nr=(c1-c0)/8 summed rows starting at
    # c0 + nr*c; stitch the full [128, 2048] from all 8 cores
    out = np.empty((NTOK, HID), np.float32)
    for c in range(NCORES):
        shard = np.asarray(results[c]["out"], np.float32)
        for (_, c0, c1, o0) in CHUNKS:
            nr = (c1 - c0) // 8
            out[c0 + nr * c: c0 + nr * c + nr] = shard[o0:o0 + nr]
    return out


def kernel(hidden_states, past_k, past_v, wq, wk, wv, wo):
    nc = _get_nc()
    in_maps = make_in_maps(hidden_states, past_k, past_v, wq, wk, wv, wo)
    res = run_bass_kernel_spmd(nc, in_maps, core_ids=list(range(NCORES)))
    return assemble_out(res.results).reshape(B, S, HID)


# revision 65
# speedup vs baseline: 1.0128x; 1.0128x over previous
"""Distributed Trainium2 kernel for decode-style multi-head attention.

Shape: B=8, S=16, H=32, D=64, HID=2048, PAST=4096 (T=4112 after concat).
Sharding: tensor-parallel over heads — each of 8 cores owns 4 heads:
  wq/wk/wv row-sharded (output features), wo column-sharded (input features),
  past KV naturally per-head; partial out-proj summed with chunked
  ReduceScatters (each core returns only its 16-row shard; the host
  assembles the full output from all 8 cores).

Per-core dataflow (all matmuls out = lhsT.T @ rhs, contract on partitions;
every matmul operand is kept at base partition 0 — base-64 operands fault
on this runtime; partition moves go through SBUF->SBUF DMA instead):
  - x / weight shards cast f32->bf16 (Pool/DVE), PE-transposed in bf16
    (1 cyc/row) -> xT, wqT/wkT/wvT [hid,256], woT [256,2048].
  - projections: qT/kT [256,128] (head-dim major), v [128,256] (token major).
  - per (b,h) pair: KV DMA'd 2KB-interleaved (partition p <- tokens
    {8p..8p+7} of each 1024-token group; consecutive 64-col slices are
    128-token tiles), K cast f32->bf16 on GPSIMD, K tiles PE-transposed in
    bf16 -> kT [64,128] tiles (base 0), PSUM->SBUF extracts on DVE (2-byte
    2x mode); scores^T = stationary kT x moving qT -> PSUM [128tok,16];
    exp on ACT -> probsT bf16; out2^T accumulated as stationary [v|1] bf16
    x moving probsT -> PSUM [65,16] (row 64 = denom); normalize via DVE
    reciprocal + gpsimd partition_broadcast + DVE multiply into attnS.
  - the emission is software-pipelined three stages deep (pre / score /
    norm) so no engine sequencer holds a cross-engine wait that blocks the
    next pair's early work, and the KV stream never stalls on tile reuse.
  - chunked out-proj (batch chunks 3/2/2/1 so the last chunk is small)
    contracts straight out of attnS against a per-head woT2 layout (no
    rebase DMAs); bf16 partials -> one wide cc_in DMA -> per-chunk
    ReduceScatter overlapped with the stream; the tail is just the last
    (16-row) RS plus the final output copy. Output is bf16 (the host casts
    back to f32 when assembling).
"""

import os

import numpy as np

import concourse.bass as bass
import concourse.mybir as mybir
import concourse.tile as tile
from concourse import bacc
from concourse.masks import make_identity
from concourse.bass_utils import run_bass_kernel_spmd

F32 = mybir.dt.float32
BF16 = mybir.dt.bfloat16

B, S, H, D = 8, 16, 32, 64
HID = H * D            # 2048
PAST = 4096
NCORES = 8
HLOC = H // NCORES     # 4 heads per core
SH = HLOC * D          # 256 local head dims
NTOK = B * S           # 128 query tokens
NT = PAST // 128       # 32 full KV tiles (8-token interleave)
SCALE = 1.0 / float(np.sqrt(D))
EXP = mybir.ActivationFunctionType.Exp

# batch chunks for the out-proj / ReduceScatter pipeline: the last chunk is a
# single batch so the post-stream tail is one small RS.
# chunk: (last_batch, row_start, row_end, out_row_start)
CHUNKS = [
    (2, 0, 48, 0),
    (4, 48, 80, 6),
    (6, 80, 112, 10),
    (7, 112, 128, 14),
]


def build_nc():
    skip_cc = os.environ.get("SKIP_CC", "0") == "1"
    kvb = int(os.environ.get("KVB", "5"))
    sbb = int(os.environ.get("SBB", "3"))
    nc = bacc.Bacc(None, target_bir_lowering=False, debug=False, num_devices=NCORES)

    hid_e = nc.declare_dram_parameter("hidden", [NTOK, HID], F32, isOutput=False)
    wq_e = nc.declare_dram_parameter("wq", [SH, HID], F32, isOutput=False)
    wk_e = nc.declare_dram_parameter("wk", [SH, HID], F32, isOutput=False)
    wv_e = nc.declare_dram_parameter("wv", [SH, HID], F32, isOutput=False)
    wo_e = nc.declare_dram_parameter("wo", [HID, SH], F32, isOutput=False)
    pk_e = nc.declare_dram_parameter("pk", [B, HLOC, PAST, D], F32, isOutput=False)
    pv_e = nc.declare_dram_parameter("pv", [B, HLOC, PAST, D], F32, isOutput=False)
    out_e = nc.declare_dram_parameter("out", [16, HID], BF16, isOutput=True)

    cc_in = nc.dram_tensor("cc_in", [NTOK, HID], BF16)
    cc_out = nc.dram_tensor("cc_out", [16, HID], BF16)

    with tile.TileContext(nc) as tc:
        with (
            tc.tile_pool(name="const", bufs=1) as constp,
            tc.tile_pool(name="pers", bufs=1) as pers,
            tc.tile_pool(name="wload", bufs=2) as wload,
            tc.tile_pool(name="kvload", bufs=kvb) as kvload,
            tc.tile_pool(name="kbfp", bufs=sbb) as kbfp,
            tc.tile_pool(name="ktp", bufs=sbb) as ktp,
            tc.tile_pool(name="vbfp", bufs=sbb) as vbfp,
            tc.tile_pool(name="probsp", bufs=sbb) as probsp,
            tc.tile_pool(name="finp", bufs=sbb) as finp,
            tc.tile_pool(name="ochp", bufs=1) as ochp,
            tc.tile_pool(name="psX", bufs=int(os.environ.get("PSX", "3")), space="PSUM") as psX,
            tc.tile_pool(name="psB", bufs=int(os.environ.get("PSB", "2")), space="PSUM") as psB,
            tc.tile_pool(name="psC", bufs=int(os.environ.get("PSC", "3")), space="PSUM") as psC,
        ):
            ident_bf = constp.tile([128, 128], BF16, tag="idb")
            make_identity(nc, ident_bf[:, :])

            # persistent per-core tensors
            xT = pers.tile([128, 16 * 128], BF16, tag="xT")
            wqT = pers.tile([128, 16 * 256], BF16, tag="wqT")
            wkT = pers.tile([128, 16 * 256], BF16, tag="wkT")
            wvT = pers.tile([128, 16 * 256], BF16, tag="wvT")
            woT2 = pers.tile([64, 4 * 2048], BF16, tag="woT2")
            qstage = pers.tile([128, 256], BF16, tag="qstage")
            kstage = pers.tile([128, 256], BF16, tag="kstage")
            qT2 = pers.tile([64, 4 * 128], BF16, tag="qT2")       # [d, hl*128+(b,s)]
            kTn = pers.tile([64, 4 * 128], BF16, tag="kTn")
            vn2 = pers.tile([16, 32 * 65], BF16, tag="vn2")
            attnS = pers.tile([64, 4 * 128], BF16, tag="attnS")   # normalized out2^T

            def load_kv(b, hl, nspl=2):
                kb = kvload.tile([128, 2048], F32, tag="kbuf")
                vb = kvload.tile([128, 2048], F32, tag="vbuf")
                ctok = PAST // nspl          # tokens per split
                ccol = 2048 // nspl          # sbuf cols per split
                gs = max(ctok // 1024, 1)    # 1024-token groups per split
                # partition p <- tokens {8p..8p+7} of each 1024-token group:
                # 2KB contiguous DMA runs; consecutive 64-col slices are still
                # 128-token tiles (for any nspl), and V uses the identical
                # interleave so probsT/v token slots stay consistent.
                for hv in range(nspl):
                    nc.sync.dma_start(
                        out=kb[:, hv * ccol:(hv + 1) * ccol].rearrange(
                            "p (g eight d) -> p g eight d", g=gs, eight=8, d=64),
                        in_=pk_e[b, hl, hv * ctok:(hv + 1) * ctok, :].rearrange(
                            "(g p eight) d -> p g eight d", g=gs, p=128, eight=8
                        ),
                    )
                    nc.sync.dma_start(
                        out=vb[:, hv * ccol:(hv + 1) * ccol].rearrange(
                            "p (g eight d) -> p g eight d", g=gs, eight=8, d=64),
                        in_=pv_e[b, hl, hv * ctok:(hv + 1) * ctok, :].rearrange(
                            "(g p eight) d -> p g eight d", g=gs, p=128, eight=8
                        ),
                    )
                return kb, vb

            pairs = [(b, hl) for b in range(B) for hl in range(HLOC)]

            # prefetch the first pairs' KV ahead of the weight loads so the
            # DMA queue starts on the big stream immediately
            PREF = int(os.environ.get("PREF", "4"))
            prefetch = {}
            for jp in range(PREF):
                prefetch[jp] = load_kv(*pairs[jp])

            # ---------------- pipeline stages ----------------
            def stage_pre(i):
                b, hl = pairs[i]
                nspl = 4 if b == B - 1 else 2
                if i in prefetch:
                    kb, vb = prefetch.pop(i)
                else:
                    kb, vb = load_kv(b, hl, nspl)
                # K cast f32 -> bf16, halves on Pool and DVE in parallel so
                # the per-pair serial chain stays short; split along the DMA
                # splits so transposes start as soon as the first split lands
                kbf = kbfp.tile([128, 2048], BF16, tag="kbf")
                cw = 2048 // nspl
                for hv in range(nspl):
                    eng = nc.gpsimd if hv % 2 == 0 else nc.vector
                    eng.tensor_copy(
                        kbf[:, hv * cw:(hv + 1) * cw], kb[:, hv * cw:(hv + 1) * cw]
                    )
                # v: cast + restride 64 -> 65 cols (ones in col 64) on ACT
                vbf = vbfp.tile([128, 32 * 65], BF16, tag="vbf")
                vbfv = vbf[:, :].rearrange("p (t c) -> p t c", t=32, c=65)
                vbv = vb[:, :].rearrange("p (t d) -> p t d", t=32, d=64)
                nc.vector.memset(vbfv[:, :, 64:65], 1.0)
                tw = 32 // nspl
                for hv in range(nspl):
                    if i < 8:
                        nc.vector.tensor_copy(
                            vbfv[:, hv * tw:(hv + 1) * tw, 0:64],
                            vbv[:, hv * tw:(hv + 1) * tw, :],
                        )
                    else:
                        nc.scalar.copy(
                            vbfv[:, hv * tw:(hv + 1) * tw, 0:64],
                            vbv[:, hv * tw:(hv + 1) * tw, :],
                        )
                # K tiles -> kT [64, 32*128] bf16, all at partitions 0:64
                kt = ktp.tile([64, 32 * 128], BF16, tag="kt")
                for gg in range(8):
                    ps = psX.tile([128, 512], BF16, tag="trp")
                    for j in range(4):
                        t = gg * 4 + j
                        nc.tensor.transpose(
                            ps[0:64, j * 128:(j + 1) * 128],
                            kbf[:, t * 64:(t + 1) * 64],
                            ident_bf[:, :],
                        )
                    nc.vector.tensor_copy(kt[:, gg * 512:(gg + 1) * 512], ps[0:64, :])
                return kt, vbf

            def stage_score(i, kt, vbf):
                b, hl = pairs[i]
                pidx = hl * 8 + b
                pt = probsp.tile([128, 544], BF16, tag="pt")
                qsl = qT2[:, hl * 128 + b * 16: hl * 128 + (b + 1) * 16]
                knsl = kTn[:, hl * 128 + b * 16: hl * 128 + (b + 1) * 16]
                for t16 in range(2):
                    ps_sc = psB.tile([128, 256], F32, tag="sc")
                    for j in range(16):
                        t = t16 * 16 + j
                        nc.tensor.matmul(
                            ps_sc[:, j * 16:(j + 1) * 16],
                            lhsT=kt[:, t * 128:(t + 1) * 128],
                            rhs=qsl,
                            start=True,
                            stop=True,
                        )
                    nc.scalar.activation(
                        pt[:, t16 * 256:(t16 + 1) * 256], ps_sc[:, :], EXP
                    )
                ps_sc = psB.tile([128, 256], F32, tag="sc")
                nc.tensor.matmul(
                    ps_sc[0:16, 0:16], lhsT=knsl, rhs=qsl, start=True, stop=True
                )
                nc.scalar.activation(pt[0:16, 512:528], ps_sc[0:16, 0:16], EXP)
                # out2^T accumulation [65, 16]; row 64 = denom
                po = psC.tile([65, 16], F32, tag="out2")
                for t in range(32):
                    nc.tensor.matmul(
                        po[:, :],
                        lhsT=vbf[:, t * 65:(t + 1) * 65],
                        rhs=pt[:, t * 16:(t + 1) * 16],
                        start=(t == 0),
                        stop=False,
                    )
                nc.tensor.matmul(
                    po[:, :],
                    lhsT=vn2[0:16, pidx * 65:(pidx + 1) * 65],
                    rhs=pt[0:16, 512:528],
                    start=False,
                    stop=True,
                )
                return po

            def stage_norm(i, po):
                b, hl = pairs[i]
                rec1 = finp.tile([1, 16], F32, tag="rec")
                nc.vector.reciprocal(rec1[:, :], po[64:65, :])
                recb = finp.tile([64, 16], F32, tag="recb")
                nc.gpsimd.partition_broadcast(recb[:, :], rec1[:, :])
                nc.vector.tensor_tensor(
                    attnS[:, hl * 128 + b * 16: hl * 128 + (b + 1) * 16],
                    po[0:64, :],
                    recb[:, :],
                    mybir.AluOpType.mult,
                )

            def stage_chunk(chunk):
                bl, c0, c1, o0 = chunk
                nr = (c1 - c0) // 8
                o1 = o0 + nr
                och = ochp.tile([48, 2048], BF16, tag="och")
                for n in range(4):
                    pso = psC.tile([c1 - c0, 512], F32, tag="out2")
                    for hl2 in range(4):
                        nc.tensor.matmul(
                            pso[:, :],
                            lhsT=attnS[:, hl2 * 128 + c0: hl2 * 128 + c1],
                            rhs=woT2[:, hl2 * 2048 + n * 512: hl2 * 2048 + (n + 1) * 512],
                            start=(hl2 == 0),
                            stop=(hl2 == 3),
                        )
                    with nc.allow_low_precision(reason="bf16 partials for RS"):
                        nc.scalar.copy(
                            och[0:c1 - c0, n * 512:(n + 1) * 512], pso[:, :]
                        )
                nc.scalar.dma_start(
                    out=(out_e[o0:o1, :] if skip_cc else cc_in[c0:c1, :]),
                    in_=(och[0:nr, :] if skip_cc else och[0:c1 - c0, :]),
                )
                if not skip_cc:
                    nc.gpsimd.collective_compute(
                        "ReduceScatter",
                        mybir.AluOpType.add,
                        replica_groups=[list(range(NCORES))],
                        ins=[cc_in[c0:c1, :].opt()],
                        outs=[cc_out[o0:o1, :].opt()],
                    )

            wcp = [0]

            def wcopy(dst, srcv):
                wcp[0] += 1
                nc.scalar.copy(dst, srcv)

            # pre-stages of the prefetched pairs come first so their Pool
            # casts and PE transposes interleave with the weight setup
            pre_st = {}
            for i in range(min(PREF, 2)):
                pre_st[i] = stage_pre(i)

            # ---------- x load + cast + transpose (all-bf16 PE path) ----------
            def wcast2(dstt, srct):
                nc.gpsimd.tensor_copy(dstt[:, 0:1024], srct[:, 0:1024])
                nc.vector.tensor_copy(dstt[:, 1024:2048], srct[:, 1024:2048])

            xsb = wload.tile([128, 2048], F32, tag="wnat")
            nc.scalar.dma_start(out=xsb[:, :], in_=hid_e[:, :])
            xsbb = kbfp.tile([128, 2048], BF16, tag="kbf")
            wcast2(xsbb, xsb)
            for r4 in range(4):
                ps = psX.tile([128, 512], BF16, tag="trp")
                for j in range(4):
                    r = r4 * 4 + j
                    nc.tensor.transpose(
                        ps[:, j * 128:(j + 1) * 128],
                        xsbb[:, r * 128:(r + 1) * 128],
                        ident_bf[:, :],
                    )
                wcopy(xT[:, r4 * 512:(r4 + 1) * 512], ps[:, :])

            # ---------- wq/wk/wv transposes ----------
            for w_e, dst in ((wq_e, wqT), (wk_e, wkT), (wv_e, wvT)):
                for p in range(2):
                    wn = wload.tile([128, 2048], F32, tag="wnat")
                    nc.scalar.dma_start(out=wn[:, :], in_=w_e[p * 128:(p + 1) * 128, :])
                    wnb = kbfp.tile([128, 2048], BF16, tag="kbf")
                    wcast2(wnb, wn)
                    for r4 in range(4):
                        ps = psX.tile([128, 512], BF16, tag="trp")
                        for j in range(4):
                            r = r4 * 4 + j
                            nc.tensor.transpose(
                                ps[:, j * 128:(j + 1) * 128],
                                wnb[:, r * 128:(r + 1) * 128],
                                ident_bf[:, :],
                            )
                        dview = dst[:, :].rearrange("q (r c) -> q r c", r=16, c=256)
                        wcopy(
                            dview[:, r4 * 4:(r4 + 1) * 4, p * 128:(p + 1) * 128],
                            ps[:, :].rearrange("q (j c) -> q j c", j=4, c=128),
                        )

            # ---------- wo transposes: woT2[d, hl*2048 + n] = wo[n, hl*64+d]
            for hhalf in range(2):
                wn = wload.tile([128, 2048], F32, tag="wnat")
                nc.scalar.dma_start(
                    out=wn[:, :],
                    in_=wo_e[:, :].rearrange("(rr p) c -> p rr c", p=128)[
                        :, hhalf * 8:(hhalf + 1) * 8, :
                    ],
                )
                wnb = kbfp.tile([128, 2048], BF16, tag="kbf")
                wcast2(wnb, wn)
                for hl in range(4):
                    for rr4 in range(2):
                        ps = psX.tile([128, 512], BF16, tag="trp")
                        for j in range(4):
                            rr_rel = rr4 * 4 + j
                            nc.tensor.transpose(
                                ps[0:64, j * 128:(j + 1) * 128],
                                wnb[:, rr_rel * 256 + hl * 64: rr_rel * 256 + (hl + 1) * 64],
                                ident_bf[:, :],
                            )
                        base = hl * 2048 + (hhalf * 8 + rr4 * 4) * 128
                        if wcp[0] % 2 == 0:
                            nc.vector.tensor_copy(woT2[:, base: base + 512], ps[0:64, :])
                        else:
                            nc.scalar.copy(woT2[:, base: base + 512], ps[0:64, :])
                        wcp[0] += 1

            # ---------- projections ----------
            for wTsrc, stg, scl in ((wqT, qstage, SCALE), (wkT, kstage, 1.0)):
                for p in range(2):
                    ps = psB.tile([128, 256], F32, tag="sc")
                    for r in range(16):
                        nc.tensor.matmul(
                            ps[:, 0:128],
                            lhsT=wTsrc[:, r * 256 + p * 128: r * 256 + (p + 1) * 128],
                            rhs=xT[:, r * 128:(r + 1) * 128],
                            start=(r == 0),
                            stop=(r == 15),
                        )
                    if scl != 1.0:
                        nc.scalar.mul(stg[:, p * 128:(p + 1) * 128], ps[:, 0:128], scl)
                    else:
                        nc.scalar.copy(stg[:, p * 128:(p + 1) * 128], ps[:, 0:128])
            # re-base to [64, hl*128 + (b,s)] layout
            for hl in range(4):
                p, hf = hl // 2, hl % 2
                nc.scalar.dma_start(
                    out=qT2[:, hl * 128:(hl + 1) * 128],
                    in_=qstage[hf * 64:(hf + 1) * 64, p * 128:(p + 1) * 128],
                )
                nc.scalar.dma_start(
                    out=kTn[:, hl * 128:(hl + 1) * 128],
                    in_=kstage[hf * 64:(hf + 1) * 64, p * 128:(p + 1) * 128],
                )

            # v projection [128 tok, 256]
            psv = psB.tile([128, 256], F32, tag="sc")
            for r in range(16):
                nc.tensor.matmul(
                    psv[:, :],
                    lhsT=xT[:, r * 128:(r + 1) * 128],
                    rhs=wvT[:, r * 256:(r + 1) * 256],
                    start=(r == 0),
                    stop=(r == 15),
                )
            vn_sb = finp.tile([128, 256], BF16, tag="vnsb")
            nc.scalar.copy(vn_sb[:, :], psv[:, :])
            vn2v = vn2[:, :].rearrange("s (pr c) -> s pr c", pr=32, c=65)
            nc.vector.memset(vn2v[:, :, 64:65], 1.0)
            for b2 in range(8):
                nc.scalar.dma_start(
                    out=vn2[:, :].rearrange("s (hl b c) -> s hl b c",
                                            hl=4, b=8, c=65)[:, :, b2, 0:64],
                    in_=vn_sb[b2 * 16:(b2 + 1) * 16, :].rearrange(
                        "s (hl d) -> s hl d", hl=4
                    ),
                )

            # ------- main attention loop (3-stage software pipeline) -------
            chunk_by_batch = {bl: (bl, c0, c1, o0) for (bl, c0, c1, o0) in CHUNKS}
            NP = len(pairs)
            po_st = {}

            def norm_and_chunk(j):
                stage_norm(j, po_st[j])
                bj, hlj = pairs[j]
                if hlj == HLOC - 1 and bj in chunk_by_batch:
                    stage_chunk(chunk_by_batch[bj])

            # norms run 2 pairs behind the front; the chunk-gating norm of
            # each (b, HLOC-1) pair runs only 1 behind so its chunk's
            # ReduceScatter fires as early as possible (the collective device
            # serializes RS's, so a late RS delays the final one).
            done_norm = set()

            def norm_upto(j):
                for k in range(j + 1):
                    if k not in done_norm and k in po_st:
                        done_norm.add(k)
                        norm_and_chunk(k)

            po_st[0] = stage_score(0, *pre_st.pop(0))
            for i in range(1, NP):
                if i not in pre_st:
                    pre_st[i] = stage_pre(i)
                po_st[i] = stage_score(i, *pre_st.pop(i))
                if pairs[i - 1][1] == HLOC - 1 and pairs[i - 1][0] >= 5:
                    norm_upto(i - 1)
                elif i >= 2:
                    norm_upto(i - 2)
                if i == NP - 1:
                    norm_upto(NP - 1)

            if not skip_cc:
                nc.sync.dma_start(out=out_e[0:14, :], in_=cc_out[0:14, :])
                nc.sync.dma_start(out=out_e[14:16, :], in_=cc_out[14:16, :])

    nc.compile()
    return nc


_CACHE = {}


def _get_nc():
    if "nc" not in _CACHE:
        _CACHE["nc"] = build_nc()
    return _CACHE["nc"]


def make_in_maps(hidden_states, past_k, past_v, wq, wk, wv, wo):
    x = np.ascontiguousarray(np.asarray(hidden_states, np.float32).reshape(NTOK, HID))
    wq = np.asarray(wq, np.float32)
    wk = np.asarray(wk, np.float32)
    wv = np.asarray(wv, np.float32)
    wo = np.asarray(wo, np.float32)
    past_k = np.asarray(past_k, np.float32)
    past_v = np.asarray(past_v, np.float32)
    in_maps = []
    for c in range(NCORES):
        rs = slice(c * SH, (c + 1) * SH)
        in_maps.append({
            "hidden": x,
            "wq": np.ascontiguousarray(wq[rs, :]),
            "wk": np.ascontiguousarray(wk[rs, :]),
            "wv": np.ascontiguousarray(wv[rs, :]),
            "wo": np.ascontiguousarray(wo[:, rs]),
            "pk": np.ascontiguousarray(past_k[:, c * HLOC:(c + 1) * HLOC]),
            "pv": np.ascontiguousarray(past_v[:, c * HLOC:(c + 1) * HLOC]),
        })
    return in_maps


def assemble_out(results):
    # each core's "out" rows are its ReduceScatter shards: for each chunk of
    # rows [c0:c1), core c holds the nr=(c1-c0)/8 summed rows starting at
    # c0 + nr*c; stitch the full [128, 2048] from all 8 cores
    out = np.empty((NTOK, HID), np.float32)
    for c in range(NCORES):
        shard = np.asarray(results[c]["out"], np.float32)
        for (_, c0, c1, o0) in CHUNKS:
            nr = (c1 - c0) // 8
            out[c0 + nr * c: c0 + nr * c + nr] = shard[o0:o0 + nr]
    return out


def kernel(hidden_states, past_k, past_v, wq, wk, wv, wo):
    nc = _get_nc()
    in_maps = make_in_maps(hidden_states, past_k, past_v, wq, wk, wv, wo)
    res = run_bass_kernel_spmd(nc, in_maps, core_ids=list(range(NCORES)))
    return assemble_out(res.results).reshape(B, S, HID)
